# revision 26
# baseline (speedup 1.0000x reference)
"""Trainium2 Bass kernel for an AttnBlock (LayerNorm -> qkv -> feature-axis
attention -> proj -> residual), sharded batch-parallel across 8 NeuronCores.

Key algebraic factoring (valid for zero qkv/norm biases, as in this problem):
the attention is over the FEATURE axis (scores [D, D], contraction over L), so
with h = LN(x) (norm_w folded into the weights host-side):

    S  = wq (h^T h) wk^T / sqrt(L)          # Gram matrix G = h^T h
    W  = softmax(S, axis=1)
    out = h (proj W wv)^T + x

This needs only 2 big [L,D]x[D,D] GEMMs (G and the final) plus four [D,D]^3
GEMMs -- ~27 GFLOP vs ~52 GFLOP for the direct q/k/v form.

Precision plan (tolerance 2e-2; measured on this input distribution):
  - G runs in fp8 e4m3 with DoubleRow perf mode (2 contraction rows per PE
    pass, 2x bf16 throughput).  The induced logit noise costs ~1.3e-2 total
    relative error.
  - Everything else (four D^3 GEMMs, final GEMM) is bf16 with fp32 PSUM.
    An all-fp8 final GEMM would push the total error over the gate.

Per-core schedule:
  A: stream x in 128-row chunks on two HWDGE queues (even chunks on sync,
     odd on scalar): LN stats (bn_stats, DVE), h chunk in bf16 (DVE),
     h8 = fp8(h) (gpsimd, SBUF->SBUF), PE-transpose h -> hT (bf16),
     G = h8^T h8 via fp8 DoubleRow (6 of 16 chains overlap the stream);
     then T2' = G wqT (bf16).
  B: S'^T = wkT^T T2' -> exp (scalar act) -> W' [j,i] bf16; row-sums via
     PE ones-matmul colsums; M = W'^T wv (/rs on copy); NT = M^T projT.
  C: out chunks = hT^T NT (bf16) + x (single DVE add); x re-reads on the
     sync queue, out writes on the scalar queue.
"""

import math
import re
from contextlib import ExitStack

import ml_dtypes
import numpy as np

import concourse.bass as bass
import concourse.mybir as mybir
import concourse.tile as tile
from concourse.vector_clock import ScopedClock, VectorClock

F32 = mybir.dt.float32
BF16 = mybir.dt.bfloat16
FP8 = mybir.dt.float8e4
AF = mybir.ActivationFunctionType
ALU = mybir.AluOpType
DR = mybir.MatmulPerfMode.DoubleRow

P = 128
D = 1024
NKT = D // P  # 8 tiles over D
LN_EPS = 1e-5


def _vc_ticks(vc):
    return [int(s) for s in re.findall(r"\d+", repr(vc))]


def _patched_drain_and_barrier(self, tick_clock, wait_clock):
    # This walrus build rejects >1 sync wait on one CTRL instruction; split
    # the kernel-tail drain into one drain per busy logical processor.
    for proc, t in enumerate(_vc_ticks(tick_clock.global_clock)):
        if t <= 0:
            continue
        d = self.nc.sync.drain()
        sub = VectorClock()
        sub.require_at_least(proc, t)
        wait_clock.add_sem_waits(d.ins, ScopedClock({None: sub}))
    self.nc.all_engine_barrier()
    popped = self.nc._tile_sem_poison_stack.pop()
    assert popped is self._sem_poison
    self.nc.clear_and_free_semaphores(list(self.sems.allocated().values()))
    self.nc.all_engine_barrier()


tile.TileContext._drain_and_barrier = _patched_drain_and_barrier

# This walrus build rejects >1 sync wait on any instruction. Spill excess
# waits onto preceding single-wait NoOps on the same engine (program order
# on the engine stream makes the split equivalent).
_MAXW = 1
_orig_commit = tile.TileContext._commit_instruction


def _commit_capped(self, inst, lazy_reg_writes=True):
    si = getattr(inst, "sync_info", None)
    eng = getattr(inst, "engine", None)
    if (si is not None and si.on_wait and len(si.on_wait) > _MAXW
            and eng is not None and eng != mybir.EngineType.Unassigned):
        waits = list(si.on_wait)
        while len(waits) > _MAXW:
            chunk, waits = waits[:_MAXW], waits[_MAXW:]
            nop = mybir.InstNoOp(
                name=f"I-{self.nc.next_id()}",
                sync_info=mybir.SyncInfo(on_wait=chunk, on_update=[]),
                bass_nofuse=True,
                engine=eng,
            )
            _orig_commit(self, nop, lazy_reg_writes=False)
        inst.sync_info = mybir.SyncInfo(on_wait=waits, on_update=si.on_update)
    return _orig_commit(self, inst, lazy_reg_writes)


tile.TileContext._commit_instruction = _commit_capped


def build_program(L, debug=False):
    nc = bass.Bass("TRN2", target_bir_lowering=False, debug=False)

    x_d = nc.dram_tensor("x", [L, D], F32, kind="ExternalInput").ap()
    wqT_d = nc.dram_tensor("wqT", [D, D], BF16, kind="ExternalInput").ap()
    wkT_d = nc.dram_tensor("wkT", [D, D], BF16, kind="ExternalInput").ap()
    wv_d = nc.dram_tensor("wv", [D, D], BF16, kind="ExternalInput").ap()
    projT_d = nc.dram_tensor("projT", [D, D], BF16, kind="ExternalInput").ap()
    ident_d = nc.dram_tensor("ident", [P, P], BF16, kind="ExternalInput").ap()
    out_d = nc.dram_tensor("out", [L, D], F32, kind="ExternalOutput").ap()
    dbg = None
    if debug:
        dbg = {
            "h8": nc.dram_tensor("dbg_h8", [P, L // P, D], FP8,
                                 kind="ExternalOutput").ap(),
            "ht": nc.dram_tensor("dbg_ht", [P, NKT, L], BF16,
                                 kind="ExternalOutput").ap(),
            "g": nc.dram_tensor("dbg_g", [P, NKT, D], BF16,
                                kind="ExternalOutput").ap(),
            "t2": nc.dram_tensor("dbg_t2", [P, NKT, D], BF16,
                                 kind="ExternalOutput").ap(),
            "w": nc.dram_tensor("dbg_w", [P, NKT, D], BF16,
                                kind="ExternalOutput").ap(),
            "m": nc.dram_tensor("dbg_m", [P, NKT, D], BF16,
                                kind="ExternalOutput").ap(),
            "nt": nc.dram_tensor("dbg_nt", [P, NKT, D], BF16,
                                 kind="ExternalOutput").ap(),
        }

    with tile.TileContext(nc) as tc:
        _emit(tc, L, x_d, wqT_d, wkT_d, wv_d, projT_d, ident_d, out_d, dbg)
    return nc


def _emit(tc, L, x_d, wqT_d, wkT_d, wv_d, projT_d, ident_d, out_d, dbg=None):
    nc = tc.nc
    NL = L // P        # 32 x-chunks
    NPR = NL // 2      # 16 DoubleRow pairs over L
    NGC = 2 * NKT      # 16 G output (row-tile, half) chains

    with ExitStack() as octx:
        const = octx.enter_context(tc.tile_pool(name="const", bufs=1))
        ident = const.tile([P, P], BF16)
        # ident is tiny and needed by the first transpose: lead the sync
        # queue with it; the big weight loads ride the gpsimd SWDGE queue so
        # the two HWDGE queues stay free for the x stream
        nc.sync.dma_start(out=ident[:], in_=ident_d[:])
        eps_t = const.tile([P, 1], F32)
        nc.vector.memset(eps_t[:], LN_EPS)
        ones_c = const.tile([P, 1], BF16)
        nc.vector.memset(ones_c[:], 1.0)
        rs_rec = const.tile([P, NKT], F32)
        nt_sb = const.tile([P, NKT, D], BF16)
        ht_sb = const.tile([P, NKT, L], BF16)
        wqT = const.tile([P, NKT, D], BF16)
        wkT = const.tile([P, NKT, D], BF16)
        t2_sb = const.tile([P, NKT, D], BF16)

        # ---------------- Phase A: LN -> h/h8, hT, G, T2' --------------
        with ExitStack() as aA:
            xin = aA.enter_context(tc.tile_pool(name="xin", bufs=6))
            stp = aA.enter_context(tc.tile_pool(name="stats", bufs=4))
            hbp = aA.enter_context(tc.tile_pool(name="hb", bufs=3))
            h8p = aA.enter_context(tc.tile_pool(name="h8", bufs=1))
            h8 = h8p.tile([P, NL, D], FP8)
            g_sb = h8p.tile([P, NKT, D], BF16)
            ptp = aA.enter_context(
                tc.tile_pool(name="ptrans", bufs=2, space="PSUM"))
            gps = aA.enter_context(
                tc.tile_pool(name="gps", bufs=6, space="PSUM"))

            def ln_chunk(c):
                xt = xin.tile([P, D], F32, tag="x", name=f"x{c}")
                qeng = nc.sync if c % 2 == 0 else nc.scalar
                qeng.dma_start(out=xt[:], in_=x_d[c * P:(c + 1) * P, :])
                st = stp.tile([P, 2, 6], F32, name=f"st{c}", tag="st")
                nc.vector.bn_stats(out=st[:, 0, :], in_=xt[:, 0:512])
                nc.vector.bn_stats(out=st[:, 1, :], in_=xt[:, 512:D])
                mv_t = stp.tile([P, 2], F32, name=f"mv{c}", tag="mv")
                nc.vector.bn_aggr(out=mv_t[:], in_=st[:])
                rstd = stp.tile([P, 1], F32, name=f"rstd{c}", tag="rstd")
                nc.scalar.activation(
                    out=rstd[:], in_=mv_t[:, 1:2], func=AF.Sqrt,
                    bias=eps_t[:], scale=1.0)
                nc.vector.reciprocal(out=rstd[:], in_=rstd[:])
                nmr = stp.tile([P, 1], F32, name=f"nmr{c}", tag="nmr")
                nc.vector.tensor_scalar(
                    out=nmr[:], in0=mv_t[:, 0:1], scalar1=rstd[:],
                    scalar2=-1.0, op0=ALU.mult, op1=ALU.mult)
                # h chunk in bf16 (scalar act) feeds the PE transposes;
                # h8 = fp8(h) (alternating DVE/scalar) feeds the Gram chains
                hb = hbp.tile([P, D], BF16, name=f"hb{c}", tag="hb")
                nc.scalar.activation(
                    out=hb[:], in_=xt[:], func=AF.Identity,
                    bias=nmr[:], scale=rstd[:])
                if c % 2 == 0:
                    nc.vector.tensor_scalar(
                        out=h8[:, c, :], in0=xt[:], scalar1=rstd[:],
                        scalar2=nmr[:], op0=ALU.mult, op1=ALU.add)
                else:
                    nc.scalar.activation(
                        out=h8[:, c, :], in_=xt[:], func=AF.Identity,
                        bias=nmr[:], scale=rstd[:])
                # PE-transpose h chunk into hT[:, kt, c*128:(c+1)*128]
                for jh in range(2):
                    pt = ptp.tile([P, 512], F32, name=f"pt{c}_{jh}", tag="pt")
                    for jj in range(4):
                        j = jh * 4 + jj
                        nc.tensor.matmul(
                            pt[:, jj * P:(jj + 1) * P],
                            hb[:, j * P:(j + 1) * P], ident[:],
                            start=True, stop=True)
                    dst = ht_sb[:, jh * 4:(jh + 1) * 4, c * P:(c + 1) * P]
                    src = pt[:].rearrange("p (j c) -> p j c", j=4)
                    if jh == 0:
                        nc.vector.tensor_copy(out=dst, in_=src)
                    else:
                        nc.scalar.copy(out=dst, in_=src)

            # G chain helpers: chain g = (d1, nn): psum [P, 512]
            g_ps = {}

            def g_mm(g, p_):
                d1, nn = divmod(g, 2)
                if p_ == 0:
                    g_ps[g] = gps.tile([P, 512], F32, tag="g", name=f"g{g}")
                nc.tensor.matmul(
                    g_ps[g][:],
                    h8[:, 2 * p_:2 * p_ + 2, d1 * P:(d1 + 1) * P],
                    h8[:, 2 * p_:2 * p_ + 2, nn * 512:(nn + 1) * 512],
                    start=(p_ == 0), stop=(p_ == NPR - 1), perf_mode=DR)

            def g_out(g):
                d1, nn = divmod(g, 2)
                dst = g_sb[:, d1, nn * 512:(nn + 1) * 512]
                if g % 2 == 0:
                    nc.vector.tensor_copy(out=dst, in_=g_ps.pop(g)[:])
                else:
                    nc.scalar.copy(out=dst, in_=g_ps.pop(g)[:])

            NEARLY = 6  # G chains overlapped with the LN stream
            for p_ in range(NPR):
                ln_chunk(2 * p_)
                ln_chunk(2 * p_ + 1)
                for g in range(NEARLY):
                    g_mm(g, p_)
                if p_ == 8:
                    # emitted mid-stream on the gpsimd engine stream (which
                    # also carries per-chunk nmr ops) so these 4MB loads do
                    # not contend with the x stream for HBM early on
                    nc.gpsimd.dma_start(
                        out=wqT[:],
                        in_=wqT_d.rearrange("(t p) n -> p t n", p=P))
                    nc.gpsimd.dma_start(
                        out=wkT[:],
                        in_=wkT_d.rearrange("(t p) n -> p t n", p=P))
            for g in range(NEARLY):
                g_out(g)
            for g in range(NEARLY, NGC):
                for p_ in range(NPR):
                    g_mm(g, p_)
                g_out(g)

            # T2'[d, i] = sum_m G[m, d-tile] wqT[m, :] (G symmetric)
            for dt_ in range(NKT):
                for nn in range(2):
                    t2p = gps.tile([P, 512], F32, tag="g", name=f"t2{dt_}{nn}")
                    for m in range(NKT):
                        nc.tensor.matmul(
                            t2p[:], g_sb[:, m, dt_ * P:(dt_ + 1) * P],
                            wqT[:, m, nn * 512:(nn + 1) * 512],
                            start=(m == 0), stop=(m == NKT - 1))
                    dst = t2_sb[:, dt_, nn * 512:(nn + 1) * 512]
                    if nn == 0:
                        nc.vector.tensor_copy(out=dst, in_=t2p[:])
                    else:
                        nc.scalar.copy(out=dst, in_=t2p[:])

            if dbg is not None:
                nc.sync.dma_start(out=dbg["h8"][:], in_=h8[:])
                nc.sync.dma_start(out=dbg["g"][:], in_=g_sb[:])

        if dbg is not None:
            nc.sync.dma_start(out=dbg["ht"][:], in_=ht_sb[:])
            nc.sync.dma_start(out=dbg["t2"][:], in_=t2_sb[:])

        # ---------------- Phase B: S' -> W' -> rs -> M -> NT -----------
        with ExitStack() as aB:
            bsb = aB.enter_context(tc.tile_pool(name="bsb", bufs=1))
            w_sb = bsb.tile([P, NKT, D], BF16)
            m_sb = bsb.tile([P, NKT, D], BF16)
            wv_sb = bsb.tile([P, NKT, D], BF16)
            nc.gpsimd.dma_start(
                out=wv_sb[:], in_=wv_d.rearrange("(t p) n -> p t n", p=P))
            projT = bsb.tile([P, NKT, D], BF16)
            nc.gpsimd.dma_start(
                out=projT[:], in_=projT_d.rearrange("(t p) n -> p t n", p=P))
            psB = aB.enter_context(
                tc.tile_pool(name="psB", bufs=4, space="PSUM"))
            rsps = aB.enter_context(
                tc.tile_pool(name="rsps", bufs=2, space="PSUM"))

            # S'[j, i] = sum_d wkT[d, j-tile]^T T2'[d, :]; exp -> W'
            for jt in range(NKT):
                for nn in range(2):
                    sp = psB.tile([P, 512], F32, tag="s", name=f"s{jt}{nn}")
                    for d_ in range(NKT):
                        nc.tensor.matmul(
                            sp[:], wkT[:, d_, jt * P:(jt + 1) * P],
                            t2_sb[:, d_, nn * 512:(nn + 1) * 512],
                            start=(d_ == 0), stop=(d_ == NKT - 1))
                    nc.scalar.activation(
                        out=w_sb[:, jt, nn * 512:(nn + 1) * 512], in_=sp[:],
                        func=AF.Exp, bias=0.0, scale=1.0)

            # rs_i = sum_j W'[j, i]; then M[i, :] = sum_j W'[j, i] wv[j, :]
            for it in range(NKT):
                rp = rsps.tile([P, 1], F32, tag="rs", name=f"rs{it}")
                for jt in range(NKT):
                    nc.tensor.matmul(
                        rp[:], w_sb[:, jt, it * P:(it + 1) * P], ones_c[:],
                        start=(jt == 0), stop=(jt == NKT - 1))
                nc.vector.reciprocal(out=rs_rec[:, it:it + 1], in_=rp[:])
                for nn in range(2):
                    mp = psB.tile([P, 512], F32, tag="s", name=f"m{it}{nn}")
                    for jt in range(NKT):
                        nc.tensor.matmul(
                            mp[:], w_sb[:, jt, it * P:(it + 1) * P],
                            wv_sb[:, jt, nn * 512:(nn + 1) * 512],
                            start=(jt == 0), stop=(jt == NKT - 1))
                    dst = m_sb[:, it, nn * 512:(nn + 1) * 512]
                    if nn == 0:
                        nc.vector.tensor_scalar(
                            out=dst, in0=mp[:], scalar1=rs_rec[:, it:it + 1],
                            scalar2=None, op0=ALU.mult)
                    else:
                        nc.scalar.activation(
                            out=dst, in_=mp[:], func=AF.Identity,
                            scale=rs_rec[:, it:it + 1])

            # NT[d, o] = sum_i M[i, d-tile] projT[i, :]
            for dt_ in range(NKT):
                for nn in range(2):
                    np_ = psB.tile([P, 512], F32, tag="s", name=f"n{dt_}{nn}")
                    for it in range(NKT):
                        nc.tensor.matmul(
                            np_[:], m_sb[:, it, dt_ * P:(dt_ + 1) * P],
                            projT[:, it, nn * 512:(nn + 1) * 512],
                            start=(it == 0), stop=(it == NKT - 1))
                    dst = nt_sb[:, dt_, nn * 512:(nn + 1) * 512]
                    if nn == 0:
                        nc.vector.tensor_copy(out=dst, in_=np_[:])
                    else:
                        nc.scalar.copy(out=dst, in_=np_[:])

            if dbg is not None:
                nc.sync.dma_start(out=dbg["w"][:], in_=w_sb[:])
                nc.sync.dma_start(out=dbg["m"][:], in_=m_sb[:])
                nc.sync.dma_start(out=dbg["nt"][:], in_=nt_sb[:])

        # ---------------- Phase C: out = hT^T NT + x -------------------
        with ExitStack() as aC:
            xrp = aC.enter_context(tc.tile_pool(name="xr", bufs=8))
            osp = aC.enter_context(tc.tile_pool(name="ost", bufs=3))
            po = aC.enter_context(
                tc.tile_pool(name="po", bufs=3, space="PSUM"))
            for c in range(NL):
                o_ps = po.tile([P, D], F32, tag="o", name=f"o{c}")
                for kt in range(NKT):
                    for nn in range(2):
                        nc.tensor.matmul(
                            o_ps[:, nn * 512:(nn + 1) * 512],
                            ht_sb[:, kt, c * P:(c + 1) * P],
                            nt_sb[:, kt, nn * 512:(nn + 1) * 512],
                            start=(kt == 0), stop=(kt == NKT - 1))
                xr = xrp.tile([P, D], F32, tag="xr", name=f"xr{c}")
                nc.sync.dma_start(out=xr[:], in_=x_d[c * P:(c + 1) * P, :])
                o_sb = osp.tile([P, D], F32, tag="ob", name=f"ob{c}")
                nc.vector.tensor_add(out=o_sb[:], in0=o_ps[:], in1=xr[:])
                nc.scalar.dma_start(
                    out=out_d[c * P:(c + 1) * P, :], in_=o_sb[:])


def make_in_map(xb, wq, wk, wv, proj, L):
    bf = ml_dtypes.bfloat16
    return {
        "x": np.ascontiguousarray(xb, np.float32),
        "wqT": np.ascontiguousarray(wq.T).astype(bf),
        "wkT": np.ascontiguousarray(wk.T).astype(bf),
        "wv": np.ascontiguousarray(wv).astype(bf),
        "projT": np.ascontiguousarray(proj.T).astype(bf),
        "ident": np.eye(P, dtype=bf),
    }


_CACHED = {}


def _get_program(L):
    if L not in _CACHED:
        _CACHED[L] = build_program(L)
    return _CACHED[L]


def _kernel_numpy(x, norm_w, norm_b, qkv_w, qkv_b, proj_w, proj_b):
    # exact fallback for the general (nonzero-bias) case; never hit by the
    # harness inputs but keeps kernel() correct for any input.
    out = np.empty_like(x)
    B, L, D_ = x.shape
    scale = np.float32(1.0 / math.sqrt(L))
    for b in range(B):
        xb = x[b]
        mu = xb.mean(-1, keepdims=True)
        var = ((xb - mu) ** 2).mean(-1, keepdims=True)
        h = (xb - mu) / np.sqrt(var + LN_EPS) * norm_w + norm_b
        qkv = h @ qkv_w.T + qkv_b
        q, k, v = qkv[:, :D_], qkv[:, D_:2 * D_], qkv[:, 2 * D_:]
        s = q.T @ (k * scale)
        s -= s.max(1, keepdims=True)
        w = np.exp(s)
        w /= w.sum(1, keepdims=True)
        a = v @ w.T
        out[b] = a @ proj_w.T + proj_b + xb
    return out


def kernel(x, norm_w, norm_b, qkv_w, qkv_b, proj_w, proj_b, _trace=False):
    from concourse.bass_utils import run_bass_kernel_spmd

    x = np.asarray(x, np.float32)
    norm_w = np.asarray(norm_w, np.float32)
    norm_b = np.asarray(norm_b, np.float32)
    qkv_w = np.asarray(qkv_w, np.float32)
    qkv_b = np.asarray(qkv_b, np.float32)
    proj_w = np.asarray(proj_w, np.float32)
    proj_b = np.asarray(proj_b, np.float32)
    B, L, D_ = x.shape
    assert D_ == D
    if (np.any(norm_b) or np.any(qkv_b) or np.any(proj_b)):
        return _kernel_numpy(x, norm_w, norm_b, qkv_w, qkv_b, proj_w, proj_b)
    # fold norm_w into the qkv weight columns; fold 1/sqrt(L) into wk
    wfold = qkv_w * norm_w[None, :]
    scale = np.float32(1.0 / math.sqrt(L))
    wq = wfold[:D]
    wk = wfold[D:2 * D] * scale
    wv = wfold[2 * D:]
    in_maps = [make_in_map(x[b], wq, wk, wv, proj_w, L) for b in range(B)]
    nc = _get_program(L)
    res = run_bass_kernel_spmd(nc, in_maps, core_ids=list(range(B)),
                               trace=_trace)
    out = np.stack([res.results[i]["out"] for i in range(B)]).astype(np.float32)
    if _trace:
        return out, res
    return out


# revision 28
# speedup vs baseline: 1.0031x; 1.0031x over previous
"""Trainium2 Bass kernel for an AttnBlock (LayerNorm -> qkv -> feature-axis
attention -> proj -> residual), sharded batch-parallel across 8 NeuronCores.

Key algebraic factoring (valid for zero qkv/norm biases, as in this problem):
the attention is over the FEATURE axis (scores [D, D], contraction over L), so
with h = LN(x) (norm_w folded into the weights host-side):

    S  = wq (h^T h) wk^T / sqrt(L)          # Gram matrix G = h^T h
    W  = softmax(S, axis=1)
    out = h (proj W wv)^T + x

This needs only 2 big [L,D]x[D,D] GEMMs (G and the final) plus four [D,D]^3
GEMMs -- ~27 GFLOP vs ~52 GFLOP for the direct q/k/v form.

Precision plan (tolerance 2e-2; measured on this input distribution):
  - G runs in fp8 e4m3 with DoubleRow perf mode (2 contraction rows per PE
    pass, 2x bf16 throughput).  The induced logit noise costs ~1.3e-2 total
    relative error.
  - Everything else (four D^3 GEMMs, final GEMM) is bf16 with fp32 PSUM.
    An all-fp8 final GEMM would push the total error over the gate.

Per-core schedule:
  A: stream x in 128-row chunks on two HWDGE queues (even chunks on sync,
     odd on scalar): LN stats (bn_stats, DVE), h chunk in bf16 (DVE),
     h8 = fp8(h) (gpsimd, SBUF->SBUF), PE-transpose h -> hT (bf16),
     G = h8^T h8 via fp8 DoubleRow (6 of 16 chains overlap the stream);
     then T2' = G wqT (bf16).
  B: S'^T = wkT^T T2' -> exp (scalar act) -> W' [j,i] bf16; row-sums via
     PE ones-matmul colsums; M = W'^T wv (/rs on copy); NT = M^T projT.
  C: out chunks = hT^T NT (bf16) + x (single DVE add); x re-reads on the
     sync queue, out writes on the scalar queue.
"""

import math
import re
from contextlib import ExitStack

import ml_dtypes
import numpy as np

import concourse.bass as bass
import concourse.mybir as mybir
import concourse.tile as tile
from concourse.vector_clock import ScopedClock, VectorClock

F32 = mybir.dt.float32
BF16 = mybir.dt.bfloat16
FP8 = mybir.dt.float8e4
AF = mybir.ActivationFunctionType
ALU = mybir.AluOpType
DR = mybir.MatmulPerfMode.DoubleRow

P = 128
D = 1024
NKT = D // P  # 8 tiles over D
LN_EPS = 1e-5


def _vc_ticks(vc):
    return [int(s) for s in re.findall(r"\d+", repr(vc))]


def _patched_drain_and_barrier(self, tick_clock, wait_clock):
    # This walrus build rejects >1 sync wait on one CTRL instruction; split
    # the kernel-tail drain into one drain per busy logical processor.
    for proc, t in enumerate(_vc_ticks(tick_clock.global_clock)):
        if t <= 0:
            continue
        d = self.nc.sync.drain()
        sub = VectorClock()
        sub.require_at_least(proc, t)
        wait_clock.add_sem_waits(d.ins, ScopedClock({None: sub}))
    self.nc.all_engine_barrier()
    popped = self.nc._tile_sem_poison_stack.pop()
    assert popped is self._sem_poison
    self.nc.clear_and_free_semaphores(list(self.sems.allocated().values()))
    self.nc.all_engine_barrier()


tile.TileContext._drain_and_barrier = _patched_drain_and_barrier

# This walrus build rejects >1 sync wait on any instruction. Spill excess
# waits onto preceding single-wait NoOps on the same engine (program order
# on the engine stream makes the split equivalent).
_MAXW = 1
_orig_commit = tile.TileContext._commit_instruction


def _commit_capped(self, inst, lazy_reg_writes=True):
    si = getattr(inst, "sync_info", None)
    eng = getattr(inst, "engine", None)
    if (si is not None and si.on_wait and len(si.on_wait) > _MAXW
            and eng is not None and eng != mybir.EngineType.Unassigned):
        waits = list(si.on_wait)
        while len(waits) > _MAXW:
            chunk, waits = waits[:_MAXW], waits[_MAXW:]
            nop = mybir.InstNoOp(
                name=f"I-{self.nc.next_id()}",
                sync_info=mybir.SyncInfo(on_wait=chunk, on_update=[]),
                bass_nofuse=True,
                engine=eng,
            )
            _orig_commit(self, nop, lazy_reg_writes=False)
        inst.sync_info = mybir.SyncInfo(on_wait=waits, on_update=si.on_update)
    return _orig_commit(self, inst, lazy_reg_writes)


tile.TileContext._commit_instruction = _commit_capped


def build_program(L, debug=False):
    nc = bass.Bass("TRN2", target_bir_lowering=False, debug=False)

    x_d = nc.dram_tensor("x", [L, D], F32, kind="ExternalInput").ap()
    wqT_d = nc.dram_tensor("wqT", [D, D], BF16, kind="ExternalInput").ap()
    wkT_d = nc.dram_tensor("wkT", [D, D], BF16, kind="ExternalInput").ap()
    wv_d = nc.dram_tensor("wv", [D, D], BF16, kind="ExternalInput").ap()
    projT_d = nc.dram_tensor("projT", [D, D], BF16, kind="ExternalInput").ap()
    ident_d = nc.dram_tensor("ident", [P, P], BF16, kind="ExternalInput").ap()
    out_d = nc.dram_tensor("out", [L, D], F32, kind="ExternalOutput").ap()
    dbg = None
    if debug:
        dbg = {
            "h8": nc.dram_tensor("dbg_h8", [P, L // P, D], FP8,
                                 kind="ExternalOutput").ap(),
            "ht": nc.dram_tensor("dbg_ht", [P, NKT, L], BF16,
                                 kind="ExternalOutput").ap(),
            "g": nc.dram_tensor("dbg_g", [P, NKT, D], BF16,
                                kind="ExternalOutput").ap(),
            "t2": nc.dram_tensor("dbg_t2", [P, NKT, D], BF16,
                                 kind="ExternalOutput").ap(),
            "w": nc.dram_tensor("dbg_w", [P, NKT, D], BF16,
                                kind="ExternalOutput").ap(),
            "m": nc.dram_tensor("dbg_m", [P, NKT, D], BF16,
                                kind="ExternalOutput").ap(),
            "nt": nc.dram_tensor("dbg_nt", [P, NKT, D], BF16,
                                 kind="ExternalOutput").ap(),
        }

    with tile.TileContext(nc) as tc:
        _emit(tc, L, x_d, wqT_d, wkT_d, wv_d, projT_d, ident_d, out_d, dbg)
    return nc


def _emit(tc, L, x_d, wqT_d, wkT_d, wv_d, projT_d, ident_d, out_d, dbg=None):
    nc = tc.nc
    NL = L // P        # 32 x-chunks
    NPR = NL // 2      # 16 DoubleRow pairs over L
    NGC = 2 * NKT      # 16 G output (row-tile, half) chains

    with ExitStack() as octx:
        const = octx.enter_context(tc.tile_pool(name="const", bufs=1))
        ident = const.tile([P, P], BF16)
        # ident is tiny and needed by the first transpose: lead the sync
        # queue with it; the big weight loads ride the gpsimd SWDGE queue so
        # the two HWDGE queues stay free for the x stream
        nc.sync.dma_start(out=ident[:], in_=ident_d[:])
        eps_t = const.tile([P, 1], F32)
        nc.vector.memset(eps_t[:], LN_EPS)
        ones_c = const.tile([P, 1], BF16)
        nc.vector.memset(ones_c[:], 1.0)
        rs_rec = const.tile([P, NKT], F32)
        nt_sb = const.tile([P, NKT, D], BF16)
        ht_sb = const.tile([P, NKT, L], BF16)
        wqT = const.tile([P, NKT, D], BF16)
        wkT = const.tile([P, NKT, D], BF16)
        t2_sb = const.tile([P, NKT, D], BF16)

        # ---------------- Phase A: LN -> h/h8, hT, G, T2' --------------
        with ExitStack() as aA:
            xin = aA.enter_context(tc.tile_pool(name="xin", bufs=6))
            stp = aA.enter_context(tc.tile_pool(name="stats", bufs=4))
            hbp = aA.enter_context(tc.tile_pool(name="hb", bufs=3))
            h8p = aA.enter_context(tc.tile_pool(name="h8", bufs=1))
            h8 = h8p.tile([P, NL, D], FP8)
            g_sb = h8p.tile([P, NKT, D], BF16)
            ptp = aA.enter_context(
                tc.tile_pool(name="ptrans", bufs=2, space="PSUM"))
            gps = aA.enter_context(
                tc.tile_pool(name="gps", bufs=6, space="PSUM"))

            def ln_chunk(c):
                xt = xin.tile([P, D], F32, tag="x", name=f"x{c}")
                qeng = nc.sync if c % 2 == 0 else nc.scalar
                qeng.dma_start(out=xt[:], in_=x_d[c * P:(c + 1) * P, :])
                st = stp.tile([P, 2, 6], F32, name=f"st{c}", tag="st")
                nc.vector.bn_stats(out=st[:, 0, :], in_=xt[:, 0:512])
                nc.vector.bn_stats(out=st[:, 1, :], in_=xt[:, 512:D])
                mv_t = stp.tile([P, 2], F32, name=f"mv{c}", tag="mv")
                nc.vector.bn_aggr(out=mv_t[:], in_=st[:])
                rstd = stp.tile([P, 1], F32, name=f"rstd{c}", tag="rstd")
                nc.scalar.activation(
                    out=rstd[:], in_=mv_t[:, 1:2], func=AF.Sqrt,
                    bias=eps_t[:], scale=1.0)
                nc.vector.reciprocal(out=rstd[:], in_=rstd[:])
                nmr = stp.tile([P, 1], F32, name=f"nmr{c}", tag="nmr")
                nc.vector.tensor_scalar(
                    out=nmr[:], in0=mv_t[:, 0:1], scalar1=rstd[:],
                    scalar2=-1.0, op0=ALU.mult, op1=ALU.mult)
                # h chunk in bf16 (scalar act) feeds the PE transposes;
                # h8 = fp8(h) (alternating DVE/scalar) feeds the Gram chains
                hb = hbp.tile([P, D], BF16, name=f"hb{c}", tag="hb")
                nc.scalar.activation(
                    out=hb[:], in_=xt[:], func=AF.Identity,
                    bias=nmr[:], scale=rstd[:])
                if c % 2 == 0:
                    nc.vector.tensor_scalar(
                        out=h8[:, c, :], in0=xt[:], scalar1=rstd[:],
                        scalar2=nmr[:], op0=ALU.mult, op1=ALU.add)
                else:
                    nc.scalar.activation(
                        out=h8[:, c, :], in_=xt[:], func=AF.Identity,
                        bias=nmr[:], scale=rstd[:])
                # PE-transpose h chunk into hT[:, kt, c*128:(c+1)*128]
                for jh in range(2):
                    pt = ptp.tile([P, 512], F32, name=f"pt{c}_{jh}", tag="pt")
                    for jj in range(4):
                        j = jh * 4 + jj
                        nc.tensor.matmul(
                            pt[:, jj * P:(jj + 1) * P],
                            hb[:, j * P:(j + 1) * P], ident[:],
                            start=True, stop=True)
                    dst = ht_sb[:, jh * 4:(jh + 1) * 4, c * P:(c + 1) * P]
                    src = pt[:].rearrange("p (j c) -> p j c", j=4)
                    if jh == 0:
                        nc.vector.tensor_copy(out=dst, in_=src)
                    else:
                        nc.scalar.copy(out=dst, in_=src)

            # G chain helpers: chain g = (d1, nn): psum [P, 512]
            g_ps = {}

            def g_mm(g, p_):
                d1, nn = divmod(g, 2)
                if p_ == 0:
                    g_ps[g] = gps.tile([P, 512], F32, tag="g", name=f"g{g}")
                nc.tensor.matmul(
                    g_ps[g][:],
                    h8[:, 2 * p_:2 * p_ + 2, d1 * P:(d1 + 1) * P],
                    h8[:, 2 * p_:2 * p_ + 2, nn * 512:(nn + 1) * 512],
                    start=(p_ == 0), stop=(p_ == NPR - 1), perf_mode=DR)

            def g_out(g):
                d1, nn = divmod(g, 2)
                dst = g_sb[:, d1, nn * 512:(nn + 1) * 512]
                if g % 2 == 0:
                    nc.vector.tensor_copy(out=dst, in_=g_ps.pop(g)[:])
                else:
                    nc.scalar.copy(out=dst, in_=g_ps.pop(g)[:])

            NEARLY = 6  # G chains overlapped with the LN stream
            for p_ in range(NPR):
                ln_chunk(2 * p_)
                ln_chunk(2 * p_ + 1)
                for g in range(NEARLY):
                    g_mm(g, p_)
                if p_ == 8:
                    # gate the 4MB weight loads on mid-stream data so they
                    # do not contend with the x stream for HBM early on
                    # (the gpsimd engine would otherwise issue them at t=0)
                    wgate = stp.tile([P, 1], F32, name="wgate", tag="wgate")
                    nc.gpsimd.tensor_copy(out=wgate[:], in_=h8[:, 2 * p_, 0:1])
                    nc.gpsimd.dma_start(
                        out=wqT[:],
                        in_=wqT_d.rearrange("(t p) n -> p t n", p=P))
                    nc.gpsimd.dma_start(
                        out=wkT[:],
                        in_=wkT_d.rearrange("(t p) n -> p t n", p=P))
            for g in range(NEARLY):
                g_out(g)
            for g in range(NEARLY, NGC):
                for p_ in range(NPR):
                    g_mm(g, p_)
                g_out(g)

            # T2'[d, i] = sum_m G[m, d-tile] wqT[m, :] (G symmetric)
            for dt_ in range(NKT):
                for nn in range(2):
                    t2p = gps.tile([P, 512], F32, tag="g", name=f"t2{dt_}{nn}")
                    for m in range(NKT):
                        nc.tensor.matmul(
                            t2p[:], g_sb[:, m, dt_ * P:(dt_ + 1) * P],
                            wqT[:, m, nn * 512:(nn + 1) * 512],
                            start=(m == 0), stop=(m == NKT - 1))
                    dst = t2_sb[:, dt_, nn * 512:(nn + 1) * 512]
                    if nn == 0:
                        nc.vector.tensor_copy(out=dst, in_=t2p[:])
                    else:
                        nc.scalar.copy(out=dst, in_=t2p[:])

            if dbg is not None:
                nc.sync.dma_start(out=dbg["h8"][:], in_=h8[:])
                nc.sync.dma_start(out=dbg["g"][:], in_=g_sb[:])

        if dbg is not None:
            nc.sync.dma_start(out=dbg["ht"][:], in_=ht_sb[:])
            nc.sync.dma_start(out=dbg["t2"][:], in_=t2_sb[:])

        # ---------------- Phase B: S' -> W' -> rs -> M -> NT -----------
        with ExitStack() as aB:
            bsb = aB.enter_context(tc.tile_pool(name="bsb", bufs=1))
            w_sb = bsb.tile([P, NKT, D], BF16)
            m_sb = bsb.tile([P, NKT, D], BF16)
            wv_sb = bsb.tile([P, NKT, D], BF16)
            nc.gpsimd.dma_start(
                out=wv_sb[:], in_=wv_d.rearrange("(t p) n -> p t n", p=P))
            projT = bsb.tile([P, NKT, D], BF16)
            nc.gpsimd.dma_start(
                out=projT[:], in_=projT_d.rearrange("(t p) n -> p t n", p=P))
            psB = aB.enter_context(
                tc.tile_pool(name="psB", bufs=4, space="PSUM"))
            rsps = aB.enter_context(
                tc.tile_pool(name="rsps", bufs=2, space="PSUM"))

            # S'[j, i] = sum_d wkT[d, j-tile]^T T2'[d, :]; exp -> W'
            for jt in range(NKT):
                for nn in range(2):
                    sp = psB.tile([P, 512], F32, tag="s", name=f"s{jt}{nn}")
                    for d_ in range(NKT):
                        nc.tensor.matmul(
                            sp[:], wkT[:, d_, jt * P:(jt + 1) * P],
                            t2_sb[:, d_, nn * 512:(nn + 1) * 512],
                            start=(d_ == 0), stop=(d_ == NKT - 1))
                    nc.scalar.activation(
                        out=w_sb[:, jt, nn * 512:(nn + 1) * 512], in_=sp[:],
                        func=AF.Exp, bias=0.0, scale=1.0)

            # rs_i = sum_j W'[j, i]; then M[i, :] = sum_j W'[j, i] wv[j, :]
            for it in range(NKT):
                rp = rsps.tile([P, 1], F32, tag="rs", name=f"rs{it}")
                for jt in range(NKT):
                    nc.tensor.matmul(
                        rp[:], w_sb[:, jt, it * P:(it + 1) * P], ones_c[:],
                        start=(jt == 0), stop=(jt == NKT - 1))
                nc.vector.reciprocal(out=rs_rec[:, it:it + 1], in_=rp[:])
                for nn in range(2):
                    mp = psB.tile([P, 512], F32, tag="s", name=f"m{it}{nn}")
                    for jt in range(NKT):
                        nc.tensor.matmul(
                            mp[:], w_sb[:, jt, it * P:(it + 1) * P],
                            wv_sb[:, jt, nn * 512:(nn + 1) * 512],
                            start=(jt == 0), stop=(jt == NKT - 1))
                    dst = m_sb[:, it, nn * 512:(nn + 1) * 512]
                    if nn == 0:
                        nc.vector.tensor_scalar(
                            out=dst, in0=mp[:], scalar1=rs_rec[:, it:it + 1],
                            scalar2=None, op0=ALU.mult)
                    else:
                        nc.scalar.activation(
                            out=dst, in_=mp[:], func=AF.Identity,
                            scale=rs_rec[:, it:it + 1])

            # NT[d, o] = sum_i M[i, d-tile] projT[i, :]
            for dt_ in range(NKT):
                for nn in range(2):
                    np_ = psB.tile([P, 512], F32, tag="s", name=f"n{dt_}{nn}")
                    for it in range(NKT):
                        nc.tensor.matmul(
                            np_[:], m_sb[:, it, dt_ * P:(dt_ + 1) * P],
                            projT[:, it, nn * 512:(nn + 1) * 512],
                            start=(it == 0), stop=(it == NKT - 1))
                    dst = nt_sb[:, dt_, nn * 512:(nn + 1) * 512]
                    if nn == 0:
                        nc.vector.tensor_copy(out=dst, in_=np_[:])
                    else:
                        nc.scalar.copy(out=dst, in_=np_[:])

            if dbg is not None:
                nc.sync.dma_start(out=dbg["w"][:], in_=w_sb[:])
                nc.sync.dma_start(out=dbg["m"][:], in_=m_sb[:])
                nc.sync.dma_start(out=dbg["nt"][:], in_=nt_sb[:])

        # ---------------- Phase C: out = hT^T NT + x -------------------
        with ExitStack() as aC:
            xrp = aC.enter_context(tc.tile_pool(name="xr", bufs=8))
            osp = aC.enter_context(tc.tile_pool(name="ost", bufs=3))
            po = aC.enter_context(
                tc.tile_pool(name="po", bufs=3, space="PSUM"))
            for c in range(NL):
                o_ps = po.tile([P, D], F32, tag="o", name=f"o{c}")
                for kt in range(NKT):
                    for nn in range(2):
                        nc.tensor.matmul(
                            o_ps[:, nn * 512:(nn + 1) * 512],
                            ht_sb[:, kt, c * P:(c + 1) * P],
                            nt_sb[:, kt, nn * 512:(nn + 1) * 512],
                            start=(kt == 0), stop=(kt == NKT - 1))
                xr = xrp.tile([P, D], F32, tag="xr", name=f"xr{c}")
                nc.sync.dma_start(out=xr[:], in_=x_d[c * P:(c + 1) * P, :])
                o_sb = osp.tile([P, D], F32, tag="ob", name=f"ob{c}")
                nc.vector.tensor_add(out=o_sb[:], in0=o_ps[:], in1=xr[:])
                nc.scalar.dma_start(
                    out=out_d[c * P:(c + 1) * P, :], in_=o_sb[:])


def make_in_map(xb, wq, wk, wv, proj, L):
    bf = ml_dtypes.bfloat16
    return {
        "x": np.ascontiguousarray(xb, np.float32),
        "wqT": np.ascontiguousarray(wq.T).astype(bf),
        "wkT": np.ascontiguousarray(wk.T).astype(bf),
        "wv": np.ascontiguousarray(wv).astype(bf),
        "projT": np.ascontiguousarray(proj.T).astype(bf),
        "ident": np.eye(P, dtype=bf),
    }


_CACHED = {}


def _get_program(L):
    if L not in _CACHED:
        _CACHED[L] = build_program(L)
    return _CACHED[L]


def _kernel_numpy(x, norm_w, norm_b, qkv_w, qkv_b, proj_w, proj_b):
    # exact fallback for the general (nonzero-bias) case; never hit by the
    # harness inputs but keeps kernel() correct for any input.
    out = np.empty_like(x)
    B, L, D_ = x.shape
    scale = np.float32(1.0 / math.sqrt(L))
    for b in range(B):
        xb = x[b]
        mu = xb.mean(-1, keepdims=True)
        var = ((xb - mu) ** 2).mean(-1, keepdims=True)
        h = (xb - mu) / np.sqrt(var + LN_EPS) * norm_w + norm_b
        qkv = h @ qkv_w.T + qkv_b
        q, k, v = qkv[:, :D_], qkv[:, D_:2 * D_], qkv[:, 2 * D_:]
        s = q.T @ (k * scale)
        s -= s.max(1, keepdims=True)
        w = np.exp(s)
        w /= w.sum(1, keepdims=True)
        a = v @ w.T
        out[b] = a @ proj_w.T + proj_b + xb
    return out


def kernel(x, norm_w, norm_b, qkv_w, qkv_b, proj_w, proj_b, _trace=False):
    from concourse.bass_utils import run_bass_kernel_spmd

    x = np.asarray(x, np.float32)
    norm_w = np.asarray(norm_w, np.float32)
    norm_b = np.asarray(norm_b, np.float32)
    qkv_w = np.asarray(qkv_w, np.float32)
    qkv_b = np.asarray(qkv_b, np.float32)
    proj_w = np.asarray(proj_w, np.float32)
    proj_b = np.asarray(proj_b, np.float32)
    B, L, D_ = x.shape
    assert D_ == D
    if (np.any(norm_b) or np.any(qkv_b) or np.any(proj_b)):
        return _kernel_numpy(x, norm_w, norm_b, qkv_w, qkv_b, proj_w, proj_b)
    # fold norm_w into the qkv weight columns; fold 1/sqrt(L) into wk
    wfold = qkv_w * norm_w[None, :]
    scale = np.float32(1.0 / math.sqrt(L))
    wq = wfold[:D]
    wk = wfold[D:2 * D] * scale
    wv = wfold[2 * D:]
    in_maps = [make_in_map(x[b], wq, wk, wv, proj_w, L) for b in range(B)]
    nc = _get_program(L)
    res = run_bass_kernel_spmd(nc, in_maps, core_ids=list(range(B)),
                               trace=_trace)
    out = np.stack([res.results[i]["out"] for i in range(B)]).astype(np.float32)
    if _trace:
        return out, res
    return out


# revision 32
# speedup vs baseline: 1.1855x; 1.1818x over previous
"""Trainium2 Bass kernel for an AttnBlock (LayerNorm -> qkv -> feature-axis
attention -> proj -> residual), sharded batch-parallel across 8 NeuronCores.

Key algebraic factoring (valid for zero qkv/norm biases, as in this problem):
the attention is over the FEATURE axis (scores [D, D], contraction over L), so
with h = LN(x) (norm_w folded into the weights host-side):

    S  = wq (h^T h) wk^T / sqrt(L)          # Gram matrix G = h^T h
    W  = softmax(S, axis=1)
    out = h (proj W wv)^T + x

This needs only 2 big [L,D]x[D,D] GEMMs (G and the final) plus four [D,D]^3
GEMMs -- ~27 GFLOP vs ~52 GFLOP for the direct q/k/v form.

Precision plan (tolerance 2e-2; measured on this input distribution):
  - G runs in fp8 e4m3 with DoubleRow perf mode (2 contraction rows per PE
    pass, 2x bf16 throughput).  The induced logit noise costs ~1.3e-2 total
    relative error.
  - Everything else (four D^3 GEMMs, final GEMM) is bf16 with fp32 PSUM.
    An all-fp8 final GEMM would push the total error over the gate.

Per-core schedule:
  A: stream x in 128-row chunks on two HWDGE queues (even chunks on sync,
     odd on scalar): LN stats (bn_stats, DVE), h chunk in bf16 (DVE),
     h8 = fp8(h) (gpsimd, SBUF->SBUF), PE-transpose h -> hT (bf16),
     G = h8^T h8 via fp8 DoubleRow (6 of 16 chains overlap the stream);
     then T2' = G wqT (bf16).
  B: S'^T = wkT^T T2' -> exp (scalar act) -> W' [j,i] bf16; row-sums via
     PE ones-matmul colsums; M = W'^T wv (/rs on copy); NT = M^T projT.
  C: out chunks = hT^T NT (bf16) + x (single DVE add); x re-reads on the
     sync queue, out writes on the scalar queue.
"""

import math
import re
from contextlib import ExitStack

import ml_dtypes
import numpy as np

import concourse.bass as bass
import concourse.mybir as mybir
import concourse.tile as tile
from concourse.vector_clock import ScopedClock, VectorClock

F32 = mybir.dt.float32
BF16 = mybir.dt.bfloat16
FP8 = mybir.dt.float8e4
AF = mybir.ActivationFunctionType
ALU = mybir.AluOpType
DR = mybir.MatmulPerfMode.DoubleRow

P = 128
D = 1024
NKT = D // P  # 8 tiles over D
LN_EPS = 1e-5


def _vc_ticks(vc):
    return [int(s) for s in re.findall(r"\d+", repr(vc))]


def _patched_drain_and_barrier(self, tick_clock, wait_clock):
    # This walrus build rejects >1 sync wait on one CTRL instruction; split
    # the kernel-tail drain into one drain per busy logical processor.
    for proc, t in enumerate(_vc_ticks(tick_clock.global_clock)):
        if t <= 0:
            continue
        d = self.nc.sync.drain()
        sub = VectorClock()
        sub.require_at_least(proc, t)
        wait_clock.add_sem_waits(d.ins, ScopedClock({None: sub}))
    self.nc.all_engine_barrier()
    popped = self.nc._tile_sem_poison_stack.pop()
    assert popped is self._sem_poison
    self.nc.clear_and_free_semaphores(list(self.sems.allocated().values()))
    self.nc.all_engine_barrier()


tile.TileContext._drain_and_barrier = _patched_drain_and_barrier

# This walrus build rejects >1 sync wait on any instruction. Spill excess
# waits onto preceding single-wait NoOps on the same engine (program order
# on the engine stream makes the split equivalent).
_MAXW = 1
_orig_commit = tile.TileContext._commit_instruction


def _commit_capped(self, inst, lazy_reg_writes=True):
    si = getattr(inst, "sync_info", None)
    eng = getattr(inst, "engine", None)
    if (si is not None and si.on_wait and len(si.on_wait) > _MAXW
            and eng is not None and eng != mybir.EngineType.Unassigned):
        waits = list(si.on_wait)
        while len(waits) > _MAXW:
            chunk, waits = waits[:_MAXW], waits[_MAXW:]
            nop = mybir.InstNoOp(
                name=f"I-{self.nc.next_id()}",
                sync_info=mybir.SyncInfo(on_wait=chunk, on_update=[]),
                bass_nofuse=True,
                engine=eng,
            )
            _orig_commit(self, nop, lazy_reg_writes=False)
        inst.sync_info = mybir.SyncInfo(on_wait=waits, on_update=si.on_update)
    return _orig_commit(self, inst, lazy_reg_writes)


tile.TileContext._commit_instruction = _commit_capped


def build_program(L, debug=False):
    nc = bass.Bass("TRN2", target_bir_lowering=False, debug=False)

    x_d = nc.dram_tensor("x", [L, D], F32, kind="ExternalInput").ap()
    wqT_d = nc.dram_tensor("wqT", [D, D], BF16, kind="ExternalInput").ap()
    wkT_d = nc.dram_tensor("wkT", [D, D], BF16, kind="ExternalInput").ap()
    wv_d = nc.dram_tensor("wv", [D, D], BF16, kind="ExternalInput").ap()
    projT_d = nc.dram_tensor("projT", [D, D], BF16, kind="ExternalInput").ap()
    ident_d = nc.dram_tensor("ident", [P, P], BF16, kind="ExternalInput").ap()
    out_d = nc.dram_tensor("out", [L, D], F32, kind="ExternalOutput").ap()
    rs_spill = nc.dram_tensor("rs_spill", [D], F32).ap()
    dbg = None
    if debug:
        dbg = {
            "h8": nc.dram_tensor("dbg_h8", [P, L // P, D], FP8,
                                 kind="ExternalOutput").ap(),
            "ht": nc.dram_tensor("dbg_ht", [P, NKT, L], BF16,
                                 kind="ExternalOutput").ap(),
            "g": nc.dram_tensor("dbg_g", [P, NKT, D], BF16,
                                kind="ExternalOutput").ap(),
            "t2": nc.dram_tensor("dbg_t2", [P, NKT, D], BF16,
                                 kind="ExternalOutput").ap(),
            "w": nc.dram_tensor("dbg_w", [P, NKT, D], BF16,
                                kind="ExternalOutput").ap(),
            "m": nc.dram_tensor("dbg_m", [P, NKT, D], BF16,
                                kind="ExternalOutput").ap(),
            "nt": nc.dram_tensor("dbg_nt", [P, NKT, D], BF16,
                                 kind="ExternalOutput").ap(),
        }

    with tile.TileContext(nc) as tc:
        _emit(tc, L, x_d, wqT_d, wkT_d, wv_d, projT_d, ident_d, out_d,
              rs_spill, dbg)
    return nc


def _emit(tc, L, x_d, wqT_d, wkT_d, wv_d, projT_d, ident_d, out_d,
          rs_spill, dbg=None):
    nc = tc.nc
    NL = L // P        # 32 x-chunks
    NPR = NL // 2      # 16 DoubleRow pairs over L
    NGC = 2 * NKT      # 16 G output (row-tile, half) chains

    with ExitStack() as octx:
        const = octx.enter_context(tc.tile_pool(name="const", bufs=1))
        ident = const.tile([P, P], BF16)
        # ident is tiny and needed by the first transpose: lead the sync
        # queue with it; the big weight loads ride the gpsimd SWDGE queue so
        # the two HWDGE queues stay free for the x stream
        nc.sync.dma_start(out=ident[:], in_=ident_d[:])
        eps_t = const.tile([P, 1], F32)
        nc.vector.memset(eps_t[:], LN_EPS)
        ones_c = const.tile([P, 1], BF16)
        nc.vector.memset(ones_c[:], 1.0)
        rs_rec = const.tile([P, NKT], F32)
        nt_sb = const.tile([P, NKT, D], BF16)
        ht_sb = const.tile([P, NKT, L], BF16)
        wqT = const.tile([P, NKT, D], BF16)
        wkT = const.tile([P, NKT, D], BF16)
        t2_sb = const.tile([P, NKT, D], BF16)

        # ---------------- Phase A: LN -> h/h8, hT, G, T2' --------------
        with ExitStack() as aA:
            xin = aA.enter_context(tc.tile_pool(name="xin", bufs=6))
            stp = aA.enter_context(tc.tile_pool(name="stats", bufs=4))
            hbp = aA.enter_context(tc.tile_pool(name="hb", bufs=3))
            h8p = aA.enter_context(tc.tile_pool(name="h8", bufs=1))
            h8 = h8p.tile([P, NL, D], FP8)
            g_sb = h8p.tile([P, NKT, D], BF16)
            ptp = aA.enter_context(
                tc.tile_pool(name="ptrans", bufs=2, space="PSUM"))
            gps = aA.enter_context(
                tc.tile_pool(name="gps", bufs=6, space="PSUM"))

            def ln_chunk(c):
                xt = xin.tile([P, D], F32, tag="x", name=f"x{c}")
                qeng = nc.sync if c % 2 == 0 else nc.scalar
                qeng.dma_start(out=xt[:], in_=x_d[c * P:(c + 1) * P, :])
                st = stp.tile([P, 2, 6], F32, name=f"st{c}", tag="st")
                nc.vector.bn_stats(out=st[:, 0, :], in_=xt[:, 0:512])
                nc.vector.bn_stats(out=st[:, 1, :], in_=xt[:, 512:D])
                mv_t = stp.tile([P, 2], F32, name=f"mv{c}", tag="mv")
                nc.vector.bn_aggr(out=mv_t[:], in_=st[:])
                rstd = stp.tile([P, 1], F32, name=f"rstd{c}", tag="rstd")
                nc.scalar.activation(
                    out=rstd[:], in_=mv_t[:, 1:2], func=AF.Sqrt,
                    bias=eps_t[:], scale=1.0)
                nc.vector.reciprocal(out=rstd[:], in_=rstd[:])
                nmr = stp.tile([P, 1], F32, name=f"nmr{c}", tag="nmr")
                nc.gpsimd.tensor_scalar(
                    out=nmr[:], in0=mv_t[:, 0:1], scalar1=rstd[:],
                    scalar2=-1.0, op0=ALU.mult, op1=ALU.mult)
                # h chunk in bf16 (scalar act) feeds the PE transposes;
                # h8 = fp8(h) is a cheap 16->8-bit DVE copy off h_bf
                hb = hbp.tile([P, D], BF16, name=f"hb{c}", tag="hb")
                nc.scalar.activation(
                    out=hb[:], in_=xt[:], func=AF.Identity,
                    bias=nmr[:], scale=rstd[:])
                nc.vector.tensor_copy(out=h8[:, c, :], in_=hb[:])
                # PE-transpose h chunk into hT[:, kt, c*128:(c+1)*128]
                for jh in range(2):
                    pt = ptp.tile([P, 512], F32, name=f"pt{c}_{jh}", tag="pt")
                    for jj in range(4):
                        j = jh * 4 + jj
                        nc.tensor.matmul(
                            pt[:, jj * P:(jj + 1) * P],
                            hb[:, j * P:(j + 1) * P], ident[:],
                            start=True, stop=True)
                    dst = ht_sb[:, jh * 4:(jh + 1) * 4, c * P:(c + 1) * P]
                    src = pt[:].rearrange("p (j c) -> p j c", j=4)
                    if jh == 0:
                        nc.vector.tensor_copy(out=dst, in_=src)
                    else:
                        nc.scalar.copy(out=dst, in_=src)

            # G chain helpers: chain g = (d1, nn): psum [P, 512]
            g_ps = {}

            def g_mm(g, p_):
                d1, nn = divmod(g, 2)
                if p_ == 0:
                    g_ps[g] = gps.tile([P, 512], F32, tag="g", name=f"g{g}")
                nc.tensor.matmul(
                    g_ps[g][:],
                    h8[:, 2 * p_:2 * p_ + 2, d1 * P:(d1 + 1) * P],
                    h8[:, 2 * p_:2 * p_ + 2, nn * 512:(nn + 1) * 512],
                    start=(p_ == 0), stop=(p_ == NPR - 1), perf_mode=DR)

            def g_out(g):
                d1, nn = divmod(g, 2)
                dst = g_sb[:, d1, nn * 512:(nn + 1) * 512]
                if g % 2 == 0:
                    nc.vector.tensor_copy(out=dst, in_=g_ps.pop(g)[:])
                else:
                    nc.scalar.copy(out=dst, in_=g_ps.pop(g)[:])

            NEARLY = 6  # G chains overlapped with the LN stream
            for p_ in range(NPR):
                ln_chunk(2 * p_)
                ln_chunk(2 * p_ + 1)
                for g in range(NEARLY):
                    g_mm(g, p_)
                if p_ == 8:
                    # gate the 4MB weight loads on mid-stream data so they
                    # do not contend with the x stream for HBM early on
                    # (the gpsimd engine would otherwise issue them at t=0)
                    wgate = stp.tile([P, 1], F32, name="wgate", tag="wgate")
                    nc.gpsimd.tensor_copy(out=wgate[:], in_=h8[:, 2 * p_, 0:1])
                    nc.gpsimd.dma_start(
                        out=wqT[:],
                        in_=wqT_d.rearrange("(t p) n -> p t n", p=P))
                    nc.gpsimd.dma_start(
                        out=wkT[:],
                        in_=wkT_d.rearrange("(t p) n -> p t n", p=P))
            for g in range(NEARLY):
                g_out(g)
            for g in range(NEARLY, NGC):
                for p_ in range(NPR):
                    g_mm(g, p_)
                g_out(g)

            # T2'[d, i] = sum_m G[m, d-tile] wqT[m, :] (G symmetric)
            for dt_ in range(NKT):
                for nn in range(2):
                    t2p = gps.tile([P, 512], F32, tag="g", name=f"t2{dt_}{nn}")
                    for m in range(NKT):
                        nc.tensor.matmul(
                            t2p[:], g_sb[:, m, dt_ * P:(dt_ + 1) * P],
                            wqT[:, m, nn * 512:(nn + 1) * 512],
                            start=(m == 0), stop=(m == NKT - 1))
                    dst = t2_sb[:, dt_, nn * 512:(nn + 1) * 512]
                    if nn == 0:
                        nc.vector.tensor_copy(out=dst, in_=t2p[:])
                    else:
                        nc.scalar.copy(out=dst, in_=t2p[:])

            if dbg is not None:
                nc.sync.dma_start(out=dbg["h8"][:], in_=h8[:])
                nc.sync.dma_start(out=dbg["g"][:], in_=g_sb[:])

        if dbg is not None:
            nc.sync.dma_start(out=dbg["ht"][:], in_=ht_sb[:])
            nc.sync.dma_start(out=dbg["t2"][:], in_=t2_sb[:])

        # ---------------- Phase B: S' -> W' -> rs -> M -> NT -----------
        with ExitStack() as aB:
            bsb = aB.enter_context(tc.tile_pool(name="bsb", bufs=1))
            w_sb = bsb.tile([P, NKT, D], BF16)
            m_sb = bsb.tile([P, NKT, D], BF16)
            wv_sb = bsb.tile([P, NKT, D], BF16)
            nc.gpsimd.dma_start(
                out=wv_sb[:], in_=wv_d.rearrange("(t p) n -> p t n", p=P))
            projT = bsb.tile([P, NKT, D], BF16)
            nc.gpsimd.dma_start(
                out=projT[:], in_=projT_d.rearrange("(t p) n -> p t n", p=P))
            psB = aB.enter_context(
                tc.tile_pool(name="psB", bufs=4, space="PSUM"))
            rsps = aB.enter_context(
                tc.tile_pool(name="rsps", bufs=2, space="PSUM"))

            # S'[j, i] = sum_d wkT[d, j-tile]^T T2'[d, :]; exp -> W'
            for jt in range(NKT):
                for nn in range(2):
                    sp = psB.tile([P, 512], F32, tag="s", name=f"s{jt}{nn}")
                    for d_ in range(NKT):
                        nc.tensor.matmul(
                            sp[:], wkT[:, d_, jt * P:(jt + 1) * P],
                            t2_sb[:, d_, nn * 512:(nn + 1) * 512],
                            start=(d_ == 0), stop=(d_ == NKT - 1))
                    nc.scalar.activation(
                        out=w_sb[:, jt, nn * 512:(nn + 1) * 512], in_=sp[:],
                        func=AF.Exp, bias=0.0, scale=1.0)

            # rs_i = sum_j W'[j, i] via two wide ones-matmul chains producing
            # a [1, D] row, reshaped to [128, 8] through a DRAM bounce
            rs_row = bsb.tile([1, D], F32)
            for nn in range(2):
                rp = rsps.tile([P, 512], F32, tag="rs", name=f"rsw{nn}")
                for jt in range(NKT):
                    nc.tensor.matmul(
                        rp[0:1, :], ones_c[:],
                        w_sb[:, jt, nn * 512:(nn + 1) * 512],
                        start=(jt == 0), stop=(jt == NKT - 1))
                nc.vector.tensor_copy(
                    out=rs_row[:, nn * 512:(nn + 1) * 512], in_=rp[0:1, :])
            nc.sync.dma_start(out=rs_spill[None, :], in_=rs_row[:])
            rs_t = bsb.tile([P, NKT], F32)
            nc.sync.dma_start(
                out=rs_t[:], in_=rs_spill.rearrange("(t p) -> p t", p=P))
            nc.vector.reciprocal(out=rs_rec[:], in_=rs_t[:])

            # M[i, :] = sum_j W'[j, i] wv[j, :]
            for it in range(NKT):
                for nn in range(2):
                    mp = psB.tile([P, 512], F32, tag="s", name=f"m{it}{nn}")
                    for jt in range(NKT):
                        nc.tensor.matmul(
                            mp[:], w_sb[:, jt, it * P:(it + 1) * P],
                            wv_sb[:, jt, nn * 512:(nn + 1) * 512],
                            start=(jt == 0), stop=(jt == NKT - 1))
                    dst = m_sb[:, it, nn * 512:(nn + 1) * 512]
                    if nn == 0:
                        nc.vector.tensor_scalar(
                            out=dst, in0=mp[:], scalar1=rs_rec[:, it:it + 1],
                            scalar2=None, op0=ALU.mult)
                    else:
                        nc.scalar.activation(
                            out=dst, in_=mp[:], func=AF.Identity,
                            scale=rs_rec[:, it:it + 1])

            # NT[d, o] = sum_i M[i, d-tile] projT[i, :]
            for dt_ in range(NKT):
                for nn in range(2):
                    np_ = psB.tile([P, 512], F32, tag="s", name=f"n{dt_}{nn}")
                    for it in range(NKT):
                        nc.tensor.matmul(
                            np_[:], m_sb[:, it, dt_ * P:(dt_ + 1) * P],
                            projT[:, it, nn * 512:(nn + 1) * 512],
                            start=(it == 0), stop=(it == NKT - 1))
                    dst = nt_sb[:, dt_, nn * 512:(nn + 1) * 512]
                    if nn == 0:
                        nc.vector.tensor_copy(out=dst, in_=np_[:])
                    else:
                        nc.scalar.copy(out=dst, in_=np_[:])

            if dbg is not None:
                nc.sync.dma_start(out=dbg["w"][:], in_=w_sb[:])
                nc.sync.dma_start(out=dbg["m"][:], in_=m_sb[:])
                nc.sync.dma_start(out=dbg["nt"][:], in_=nt_sb[:])

        # ---------------- Phase C: out = hT^T NT + x -------------------
        with ExitStack() as aC:
            xrp = aC.enter_context(tc.tile_pool(name="xr", bufs=8))
            osp = aC.enter_context(tc.tile_pool(name="ost", bufs=3))
            po = aC.enter_context(
                tc.tile_pool(name="po", bufs=3, space="PSUM"))
            for c in range(NL):
                o_ps = po.tile([P, D], F32, tag="o", name=f"o{c}")
                for kt in range(NKT):
                    for nn in range(2):
                        nc.tensor.matmul(
                            o_ps[:, nn * 512:(nn + 1) * 512],
                            ht_sb[:, kt, c * P:(c + 1) * P],
                            nt_sb[:, kt, nn * 512:(nn + 1) * 512],
                            start=(kt == 0), stop=(kt == NKT - 1))
                xr = xrp.tile([P, D], F32, tag="xr", name=f"xr{c}")
                nc.sync.dma_start(out=xr[:], in_=x_d[c * P:(c + 1) * P, :])
                o_sb = osp.tile([P, D], F32, tag="ob", name=f"ob{c}")
                nc.vector.tensor_add(out=o_sb[:], in0=o_ps[:], in1=xr[:])
                nc.scalar.dma_start(
                    out=out_d[c * P:(c + 1) * P, :], in_=o_sb[:])


def make_in_map(xb, wq, wk, wv, proj, L):
    bf = ml_dtypes.bfloat16
    return {
        "x": np.ascontiguousarray(xb, np.float32),
        "wqT": np.ascontiguousarray(wq.T).astype(bf),
        "wkT": np.ascontiguousarray(wk.T).astype(bf),
        "wv": np.ascontiguousarray(wv).astype(bf),
        "projT": np.ascontiguousarray(proj.T).astype(bf),
        "ident": np.eye(P, dtype=bf),
    }


_CACHED = {}


def _get_program(L):
    if L not in _CACHED:
        _CACHED[L] = build_program(L)
    return _CACHED[L]


def _kernel_numpy(x, norm_w, norm_b, qkv_w, qkv_b, proj_w, proj_b):
    # exact fallback for the general (nonzero-bias) case; never hit by the
    # harness inputs but keeps kernel() correct for any input.
    out = np.empty_like(x)
    B, L, D_ = x.shape
    scale = np.float32(1.0 / math.sqrt(L))
    for b in range(B):
        xb = x[b]
        mu = xb.mean(-1, keepdims=True)
        var = ((xb - mu) ** 2).mean(-1, keepdims=True)
        h = (xb - mu) / np.sqrt(var + LN_EPS) * norm_w + norm_b
        qkv = h @ qkv_w.T + qkv_b
        q, k, v = qkv[:, :D_], qkv[:, D_:2 * D_], qkv[:, 2 * D_:]
        s = q.T @ (k * scale)
        s -= s.max(1, keepdims=True)
        w = np.exp(s)
        w /= w.sum(1, keepdims=True)
        a = v @ w.T
        out[b] = a @ proj_w.T + proj_b + xb
    return out


def kernel(x, norm_w, norm_b, qkv_w, qkv_b, proj_w, proj_b, _trace=False):
    from concourse.bass_utils import run_bass_kernel_spmd

    x = np.asarray(x, np.float32)
    norm_w = np.asarray(norm_w, np.float32)
    norm_b = np.asarray(norm_b, np.float32)
    qkv_w = np.asarray(qkv_w, np.float32)
    qkv_b = np.asarray(qkv_b, np.float32)
    proj_w = np.asarray(proj_w, np.float32)
    proj_b = np.asarray(proj_b, np.float32)
    B, L, D_ = x.shape
    assert D_ == D
    if (np.any(norm_b) or np.any(qkv_b) or np.any(proj_b)):
        return _kernel_numpy(x, norm_w, norm_b, qkv_w, qkv_b, proj_w, proj_b)
    # fold norm_w into the qkv weight columns; fold 1/sqrt(L) into wk
    wfold = qkv_w * norm_w[None, :]
    scale = np.float32(1.0 / math.sqrt(L))
    wq = wfold[:D]
    wk = wfold[D:2 * D] * scale
    wv = wfold[2 * D:]
    in_maps = [make_in_map(x[b], wq, wk, wv, proj_w, L) for b in range(B)]
    nc = _get_program(L)
    res = run_bass_kernel_spmd(nc, in_maps, core_ids=list(range(B)),
                               trace=_trace)
    out = np.stack([res.results[i]["out"] for i in range(B)]).astype(np.float32)
    if _trace:
        return out, res
    return out
